# revision 1
# baseline (speedup 1.0000x reference)
"""BitAstroGPT forward pass on 8 TRN2 NeuronCores.

Sharding: sequence-parallel (4 chunks of 512 tokens per sequence) x
batch-parallel (2 sequences). Core c owns batch c//4, token chunk c%4.
Per layer, one AllGather (groups [[0..3],[4..7]]) shares K (feature-major)
and V (token-major) in bf16; attention runs full-context with host-provided
multiplicative causal masks so the SPMD program is identical on all cores.

BitNet ternary quantization is exact in bf16; the per-matrix gamma scales are
folded into scalar immediates (exp scale, output scales). Softmax runs
without max-subtraction (|scale*S| ~ 1.5 for this model) and the denominator
comes free from a ones-column appended to V in SBUF.
"""
import os
import numpy as np
import ml_dtypes

BF = ml_dtypes.bfloat16
V, B, T, D, L, H = 32000, 2, 2048, 1024, 4, 16
HD = 64
HID = 2730
HPAD = 2816           # 22 * 128
NMH = HPAD // 128     # 22
TC = 512              # tokens per core
NET = D // 128        # 8 feature tiles
NKT = T // 128        # 16 key tiles
NVT = V // 128        # 250 vocab tiles
EPS = 1e-6
GROUPS = [[0, 1, 2, 3], [4, 5, 6, 7]]

_cache = {}


def _quant(w):
    gamma = max(np.float32(np.mean(np.abs(w), dtype=np.float32)), np.float32(1e-5))
    tern = np.clip(np.round(np.float32(w) / gamma), -1.0, 1.0).astype(np.float32)
    return tern, float(gamma)


def _rope_tables():
    inv_freq = 1.0 / (10000.0 ** (np.arange(0, HD, 2, dtype=np.float32) / HD))
    t = np.arange(T, dtype=np.float32)
    freqs = np.einsum("i,j->ij", t, inv_freq)
    emb = np.concatenate([freqs, freqs], axis=-1)  # [T, 64]
    return np.cos(emb).astype(np.float32), np.sin(emb).astype(np.float32)


def _rot_lhs():
    # rot(q) = M @ q per 64-block; lhsT[e_in, e_out] = M[e_out, e_in]
    M = np.zeros((128, 128), np.float32)
    for blk in range(2):
        o = blk * 64
        for j in range(32):
            M[o + j, o + j + 32] = -1.0
            M[o + j + 32, o + j] = 1.0
    return np.ascontiguousarray(M.T).astype(BF)


def _build(scalars):
    ABL = set(os.environ.get("KERNEL_ABLATE", "").split(","))
    G = int(os.environ.get("KERNEL_G", "4"))
    RING2 = bool(int(os.environ.get("KERNEL_RING2", "0")))
    ANYC = bool(int(os.environ.get("KERNEL_ANYCOPY", "1")))
    import concourse.bacc as bacc
    import concourse.mybir as mybir
    import concourse.tile as tile

    F32 = mybir.dt.float32
    BF16 = mybir.dt.bfloat16
    AF = mybir.ActivationFunctionType
    OP = mybir.AluOpType
    es_l, vo_l, sil_l, m23_l = scalars

    nc = bacc.Bacc("TRN2", target_bir_lowering=False, debug=False, num_devices=8)

    xT0 = nc.dram_tensor("xT0", [D, TC], F32, kind="ExternalInput")
    cosf = nc.dram_tensor("cosf", [128, TC], F32, kind="ExternalInput")
    sinf = nc.dram_tensor("sinf", [128, TC], F32, kind="ExternalInput")
    maskT = nc.dram_tensor("maskT", [T, TC], BF16, kind="ExternalInput")
    rlhs = nc.dram_tensor("rlhs", [128, 128], BF16, kind="ExternalInput")
    g1s = nc.dram_tensor("g1s", [128, L * NET], F32, kind="ExternalInput")
    g2s = nc.dram_tensor("g2s", [128, L * NET], F32, kind="ExternalInput")
    gfs = nc.dram_tensor("gfs", [128, NET], F32, kind="ExternalInput")
    wq = nc.dram_tensor("wq", [L, D, D], BF16, kind="ExternalInput")
    wk = nc.dram_tensor("wk", [L, D, D], BF16, kind="ExternalInput")
    wv = nc.dram_tensor("wv", [L, D, D], BF16, kind="ExternalInput")
    wo = nc.dram_tensor("wo", [L, D, D], BF16, kind="ExternalInput")
    w1t = nc.dram_tensor("w1t", [L, D, HPAD], BF16, kind="ExternalInput")
    w3t = nc.dram_tensor("w3t", [L, D, HPAD], BF16, kind="ExternalInput")
    w2t = nc.dram_tensor("w2t", [L, HPAD, D], BF16, kind="ExternalInput")
    wlm = nc.dram_tensor("wlm", [D, V], BF16, kind="ExternalInput")
    logitsT = nc.dram_tensor("logitsT", [V, TC], F32, kind="ExternalOutput")

    with tile.TileContext(nc) as tc:
        with (
            tc.tile_pool(name="sb", bufs=3) as sb,
            tc.tile_pool(name="ps", bufs=int(os.environ.get("KERNEL_ACCB", "5")), space="PSUM") as ps,
            tc.tile_pool(name="dram", bufs=1, space="DRAM") as dram,
        ):
            dma2 = nc.scalar if RING2 else nc.sync
            anyeng = nc.any if ANYC else nc.vector
            # ---- persistent constants ----
            ones_bf = sb.tile([128, 128], BF16, tag="ones", name="ones_bf", bufs=1)
            nc.vector.memset(ones_bf[:], 1.0)
            ones32 = sb.tile([128, 128], F32, tag="ones32", name="ones32", bufs=1)
            nc.vector.memset(ones32[:], 1.0)
            rlhs_sb = sb.tile([128, 128], BF16, tag="rlhs", name="rlhs_sb", bufs=1)
            nc.sync.dma_start(rlhs_sb[:], rlhs[:])
            cos_sb = sb.tile([128, TC], F32, tag="cos", name="cos_sb", bufs=1)
            nc.sync.dma_start(cos_sb[:], cosf[:])
            sin_sb = sb.tile([128, TC], F32, tag="sin", name="sin_sb", bufs=1)
            nc.sync.dma_start(sin_sb[:], sinf[:])
            mask_sb = sb.tile([128, NKT, TC], BF16, tag="mask", name="mask_sb", bufs=1)
            nc.sync.dma_start(
                mask_sb[:], maskT[:].rearrange("(kt p) t -> p kt t", p=128))
            g1_sb = sb.tile([128, L * NET], F32, tag="g1", name="g1_sb", bufs=1)
            nc.sync.dma_start(g1_sb[:], g1s[:])
            g2_sb = sb.tile([128, L * NET], F32, tag="g2", name="g2_sb", bufs=1)
            nc.sync.dma_start(g2_sb[:], g2s[:])
            gf_sb = sb.tile([128, NET], F32, tag="gf", name="gf_sb", bufs=1)
            nc.sync.dma_start(gf_sb[:], gfs[:])

            eps_sb = sb.tile([1, 1], F32, tag="eps", name="eps_sb", bufs=1)
            nc.vector.memset(eps_sb[:], EPS)

            x_big = sb.tile([128, NET, TC], F32, tag="x", name="x_big", bufs=1)
            for i in range(NET):
                nc.sync.dma_start(x_big[:, i, :], xT0[i * 128:(i + 1) * 128, :])

            k_all = sb.tile([128, 4 * NET, TC], BF16, tag="kall", name="k_all", bufs=1)
            q_z = sb.tile([128, 16, TC], BF16, tag="qz", name="q_z", bufs=1)
            nc.vector.memset(q_z[:], 0.0)
            v_pad = sb.tile([128, NKT, 16 * 65], BF16, tag="vpad", name="v_pad", bufs=1)
            ones_view = v_pad[:].rearrange("p kt (h c) -> p kt h c", c=65)[:, :, :, 64:65]
            nc.vector.memset(ones_view, 1.0)

            # ---- helpers ----
            def proj(wslice, rhs_tiles, n_m, epi, nk=None, rhs_slice=None):
                """out[m,:] = sum_k wslice(k)[:, m*128:(m+1)*128].T @ rhs[k]"""
                nk = len(rhs_tiles) if nk is None else nk

                def wslice2(kp, kn, g0, gm):
                    return wslice(kp, g0, gm, kn).rearrange(
                        "(k p) m -> p k m", p=128)
                for g0 in range(0, n_m, G):
                    gm = min(G, n_m - g0)
                    accs = [ps.tile([128, TC], F32, tag="acc", name=f"acc{mi}")
                            for mi in range(gm)]
                    for kp in range(0, nk, 2):
                        kn = min(2, nk - kp)
                        w_sb = sb.tile([128, kn, gm * 128], BF16, tag="w",
                                       name="w_sb", bufs=3)
                        nc.sync.dma_start(w_sb[:], wslice2(kp, kn, g0, gm))
                        for ki in range(kn):
                            k = kp + ki
                            for mi in range(gm):
                                nc.tensor.matmul(
                                    accs[mi][:],
                                    w_sb[:, ki, mi * 128:(mi + 1) * 128],
                                    rhs_tiles[k][:], start=(k == 0),
                                    stop=(k == nk - 1))
                    for mi in range(gm):
                        epi(g0 + mi, accs[mi])

            def rmsnorm(g_base, g_off):
                ssum = ps.tile([1, TC], F32, tag="acc", name="ssum")
                for i in range(NET):
                    x2 = sb.tile([128, TC], BF16, tag="x2", name="x2", bufs=2)
                    nc.vector.tensor_mul(x2[:], x_big[:, i, :], x_big[:, i, :])
                    nc.tensor.matmul(ssum[:], ones_bf[:, 0:1], x2[:],
                                     start=(i == 0), stop=(i == NET - 1))
                sq = sb.tile([1, TC], F32, tag="nrm", name="sq", bufs=2)
                nc.scalar.activation(sq[:], ssum[:], AF.Sqrt, bias=eps_sb[0:1, 0:1],
                                     scale=1.0 / D)
                inv = sb.tile([1, TC], F32, tag="nrm", name="inv", bufs=2)
                nc.vector.reciprocal(inv[:], sq[:])
                rsig = ps.tile([128, TC], F32, tag="acc", name="rsig")
                nc.tensor.matmul(rsig[:], ones32[0:1, :], inv[:], start=True, stop=True)
                outs = []
                for i in range(NET):
                    o = sb.tile([128, TC], BF16, tag="hn", name="hn", bufs=8)
                    nc.vector.scalar_tensor_tensor(
                        o[:], x_big[:, i, :], g_base[:, g_off + i:g_off + i + 1],
                        rsig[:], OP.mult, OP.mult)
                    outs.append(o)
                return outs

            def rope_tile(src, sink):
                rp = ps.tile([128, TC], F32, tag="acc", name="rotp")
                nc.tensor.matmul(rp[:], rlhs_sb[:], src[:], start=True, stop=True)
                t1 = sb.tile([128, TC], F32, tag="rt", name="rt1", bufs=2)
                nc.vector.tensor_mul(t1[:], src[:], cos_sb[:])
                t2 = sb.tile([128, TC], F32, tag="rt", name="rt2", bufs=2)
                nc.vector.tensor_mul(t2[:], rp[:], sin_sb[:])
                return sink(t1, t2)

            # ---- layers ----
            for l in range(L):
                hq = rmsnorm(g1_sb, l * NET)

                ag_in = dram.tile([2 * D, TC], BF16, tag="agin", name="ag_in")
                agk_out = dram.tile([4 * D, TC], BF16, tag="agko", name="agk_out")
                agv_out = dram.tile([4 * D, TC], BF16, tag="agvo", name="agv_out")

                # k projection with fused rope, straight into ag_in
                def k_epi(m, acc):
                    t = sb.tile([128, TC], BF16, tag="ksb", name="ksb", bufs=3)
                    anyeng.tensor_copy(t[:], acc[:])
                    def sink(t1, t2, _m=m):
                        kr = sb.tile([128, TC], BF16, tag="kr", name="kr", bufs=3)
                        nc.vector.tensor_add(kr[:], t1[:], t2[:])
                        dma2.dma_start(ag_in[_m * 128:(_m + 1) * 128, :], kr[:])
                    rope_tile(t, sink)
                proj(lambda k, g0, gm, kn=1, _l=l: wk[_l, k * 128:(k + kn) * 128,
                                                g0 * 128:(g0 + gm) * 128],
                     hq, NET, k_epi)

                nc.gpsimd.collective_compute(
                    "AllGather", mybir.AluOpType.bypass, replica_groups=GROUPS,
                    ins=[ag_in[0:D, :]], outs=[agk_out[:]])

                # v projection, token-major, straight into ag_in
                agin_v = ag_in[D:2 * D, :].rearrange("(t two) c -> t (two c)", two=2)
                for half in range(2):
                    for tg in range(2):
                        vaccs = [ps.tile([128, TC], F32, tag="acc", name=f"vacc{tt}")
                                 for tt in range(2)]
                        for k in range(NET):
                            wv_sb = sb.tile([128, TC], BF16, tag="w",
                                            name="wv_sb", bufs=3)
                            nc.sync.dma_start(
                                wv_sb[:], wv[l, k * 128:(k + 1) * 128,
                                             half * 512:(half + 1) * 512])
                            for tt in range(2):
                                t_abs = tg * 2 + tt
                                nc.tensor.matmul(
                                    vaccs[tt][:],
                                    hq[k][:, t_abs * 128:(t_abs + 1) * 128],
                                    wv_sb[:], start=(k == 0), stop=(k == NET - 1))
                        for tt in range(2):
                            t_abs = tg * 2 + tt
                            vsb = sb.tile([128, TC], BF16, tag="vsb",
                                          name="vsb", bufs=2)
                            anyeng.tensor_copy(vsb[:], vaccs[tt][:])
                            dma2.dma_start(
                                agin_v[t_abs * 128:(t_abs + 1) * 128,
                                       half * 512:(half + 1) * 512], vsb[:])

                if "nocoll" not in ABL:
                    nc.gpsimd.collective_compute(
                        "AllGather", mybir.AluOpType.bypass, replica_groups=GROUPS,
                        ins=[ag_in[D:2 * D, :]], outs=[agv_out[:]])

                # q projection (overlaps the AllGather) with fused rope
                def q_epi(m, acc):
                    t = sb.tile([128, TC], BF16, tag="qsb", name="qsb", bufs=3)
                    anyeng.tensor_copy(t[:], acc[:])
                    def sink(t1, t2, _m=m):
                        nc.vector.tensor_add(q_z[0:64, 2 * _m, :],
                                             t1[0:64, :], t2[0:64, :])
                        nc.vector.tensor_add(q_z[64:128, 2 * _m + 1, :],
                                             t1[64:128, :], t2[64:128, :])
                    rope_tile(t, sink)
                proj(lambda k, g0, gm, kn=1, _l=l: wq[_l, k * 128:(k + kn) * 128,
                                                g0 * 128:(g0 + gm) * 128],
                     hq, NET, q_epi)

                # unpack AG: K feature-major + V token-major w/ ones interleave
                for r in range(4):
                    dma2.dma_start(
                        k_all[:, r * NET:(r + 1) * NET, :],
                        agk_out[r * D:(r + 1) * D, :]
                        .rearrange("(e p) t -> p e t", p=128))
                    vblk = agv_out[r * D:(r + 1) * D, :] \
                        .rearrange("(t two) c -> t (two c)", two=2)  # [512, 1024]
                    for h in range(16):
                        dma2.dma_start(
                            v_pad[:, r * 4:(r + 1) * 4, h * 65:h * 65 + 64],
                            vblk[:, h * 64:(h + 1) * 64]
                            .rearrange("(tt p) e -> p tt e", p=128))

                # attention per head
                y_tiles = [sb.tile([128, TC], BF16, tag="yall", name=f"yall{i}",
                                   bufs=8) for i in range(NET)]
                for h in range(16 if "noattn" not in ABL else 0):
                    et = h // 2
                    y_aug = ps.tile([65, TC], F32, tag="y", name="y_aug", bufs=int(os.environ.get("KERNEL_YB", "1")))
                    for kt in range(NKT):
                        r, ktl = kt // 4, kt % 4
                        ksl = k_all[:, r * NET + et, ktl * 128:(ktl + 1) * 128]
                        s_ps = ps.tile([128, TC], F32, tag="s", name="s_ps", bufs=int(os.environ.get("KERNEL_SB", "2")))
                        nc.tensor.matmul(s_ps[:], ksl, q_z[:, h, :],
                                         start=True, stop=True)
                        p_sb = sb.tile([128, TC], BF16, tag="p", name="p_sb", bufs=5)
                        nc.scalar.activation(p_sb[:], s_ps[:], AF.Exp, scale=es_l[l])
                        nc.vector.tensor_mul(p_sb[:], p_sb[:], mask_sb[:, kt, :])
                        nc.tensor.matmul(
                            y_aug[:], v_pad[:, kt, h * 65:(h + 1) * 65], p_sb[:],
                            start=(kt == 0), stop=(kt == NKT - 1))
                    rec = sb.tile([1, TC], F32, tag="rec", name="rec", bufs=2)
                    nc.vector.reciprocal(rec[0:1, :], y_aug[64:65, :])
                    rh_sb = sb.tile([64, TC], F32, tag="rh", name="rh_sb", bufs=2)
                    nc.gpsimd.partition_broadcast(rh_sb[:], rec[0:1, :])
                    half = h % 2
                    nc.vector.tensor_mul(
                        y_tiles[et][half * 64:(half + 1) * 64, :],
                        y_aug[0:64, :], rh_sb[:])

                # o_proj + residual
                def o_epi(m, acc, _l=l):
                    nc.vector.scalar_tensor_tensor(
                        x_big[:, m, :], acc[:], vo_l[_l], x_big[:, m, :],
                        OP.mult, OP.add)
                proj(lambda k, g0, gm, kn=1, _l=l: wo[_l, k * 128:(k + kn) * 128,
                                                g0 * 128:(g0 + gm) * 128],
                     y_tiles, NET, o_epi)

                # ---- MLP ----
                if "nomlp" in ABL:
                    continue
                hm = rmsnorm(g2_sb, l * NET)
                prods = []
                for g0 in range(0, NMH, G):
                    gm = min(G, NMH - g0)
                    s_tiles, b_tiles = [], []
                    def s_epi(m, acc, _l=l):
                        t = sb.tile([128, TC], BF16, tag="asb", name="asb", bufs=4)
                        nc.scalar.activation(t[:], acc[:], AF.Silu, scale=sil_l[_l])
                        s_tiles.append(t)
                    def b_epi(m, acc):
                        t = sb.tile([128, TC], BF16, tag="bsb", name="bsb", bufs=4)
                        anyeng.tensor_copy(t[:], acc[:])
                        b_tiles.append(t)
                    proj(lambda k, gg0, gm_, kn=1, _l=l, _g0=g0:
                         w1t[_l, k * 128:(k + kn) * 128,
                             _g0 * 128:(_g0 + gm_) * 128],
                         hm, gm, s_epi)
                    proj(lambda k, gg0, gm_, kn=1, _l=l, _g0=g0:
                         w3t[_l, k * 128:(k + kn) * 128,
                             _g0 * 128:(_g0 + gm_) * 128],
                         hm, gm, b_epi)
                    for mi in range(gm):
                        pr = sb.tile([128, TC], BF16, tag="prod", name="prod", bufs=22)
                        nc.vector.tensor_mul(pr[:], s_tiles[mi][:], b_tiles[mi][:])
                        prods.append(pr)

                def w2_epi(m, acc, _l=l):
                    nc.vector.scalar_tensor_tensor(
                        x_big[:, m, :], acc[:], m23_l[_l], x_big[:, m, :],
                        OP.mult, OP.add)
                proj(lambda k, g0, gm, kn=1, _l=l: w2t[_l, k * 128:(k + kn) * 128,
                                                 g0 * 128:(g0 + gm) * 128],
                     prods, NET, w2_epi)

            # ---- final norm + lm head ----
            hf = rmsnorm(gf_sb, 0)

            if "nolm" in ABL:
                NVT_eff = 8
            else:
                NVT_eff = NVT
            def lm_epi(m, acc):
                lg = sb.tile([128, TC], F32, tag="lg", name="lg", bufs=2)
                anyeng.tensor_copy(lg[:], acc[:])
                dma2.dma_start(logitsT[m * 128:(m + 1) * 128, :], lg[:])
            proj(lambda k, g0, gm, kn=1: wlm[k * 128:(k + kn) * 128,
                                       g0 * 128:(g0 + gm) * 128],
                 hf, NVT_eff, lm_epi)

    nc.compile()
    return nc


def _prep(inputs):
    """Host-side prep: quantization, layouts, per-core in_maps."""
    idx = np.asarray(inputs["idx"])
    emb = np.asarray(inputs["emb"], np.float32)

    qw = {}
    gam = {}
    for name in ["Wq", "Wk", "Wv", "Wo", "W1", "W3", "W2"]:
        W = np.asarray(inputs[name], np.float32)
        qw[name] = []
        gam[name] = []
        for l in range(L):
            t, g = _quant(W[l])
            qw[name].append(t)
            gam[name].append(g)

    es_l = tuple(gam["Wq"][l] * gam["Wk"][l] / np.sqrt(HD) for l in range(L))
    vo_l = tuple(gam["Wv"][l] * gam["Wo"][l] for l in range(L))
    sil_l = tuple(gam["W1"][l] for l in range(L))
    m23_l = tuple(gam["W2"][l] * gam["W3"][l] for l in range(L))
    scalars = (es_l, vo_l, sil_l, m23_l)

    # shared weight arrays (transposed to lhsT layout [K, M])
    wq_a = np.stack([qw["Wq"][l].T for l in range(L)]).astype(BF)
    wk_a = np.stack([qw["Wk"][l].T for l in range(L)]).astype(BF)
    wv_a = np.stack([qw["Wv"][l].T for l in range(L)]).astype(BF)
    wo_a = np.stack([qw["Wo"][l].T for l in range(L)]).astype(BF)
    w1_a = np.zeros((L, D, HPAD), BF)
    w3_a = np.zeros((L, D, HPAD), BF)
    w2_a = np.zeros((L, HPAD, D), BF)
    for l in range(L):
        w1_a[l, :, :HID] = qw["W1"][l].T.astype(BF)
        w3_a[l, :, :HID] = qw["W3"][l].T.astype(BF)
        w2_a[l, :HID, :] = qw["W2"][l].T.astype(BF)
    wlm_a = np.ascontiguousarray(np.asarray(inputs["Wlm"], np.float32).T).astype(BF)

    def gcol(g):  # [L, D] -> [128, L*8]
        return np.ascontiguousarray(
            np.asarray(g, np.float32).reshape(-1, NET, 128).transpose(2, 0, 1)
            .reshape(128, -1))
    g1s_a = gcol(inputs["g1"])
    g2s_a = gcol(inputs["g2"])
    gfs_a = gcol(np.asarray(inputs["gf"], np.float32)[None])
    rlhs_a = _rot_lhs()

    cos, sin = _rope_tables()
    row = np.tile(np.arange(HD), 2)

    in_maps = []
    for c in range(8):
        b, j = c // 4, c % 4
        tsl = slice(j * TC, (j + 1) * TC)
        toks = idx[b, tsl]
        x0 = np.ascontiguousarray(emb[toks].T)  # [D, TC] f32
        cos_fm = np.ascontiguousarray(cos[tsl][:, row].T)
        sin_fm = np.ascontiguousarray(sin[tsl][:, row].T)
        tq = np.arange(j * TC, (j + 1) * TC)[None, :]
        tk = np.arange(T)[:, None]
        mask = (tk <= tq).astype(np.float32).astype(BF)
        in_maps.append({
            "xT0": x0, "cosf": cos_fm, "sinf": sin_fm, "maskT": mask,
            "rlhs": rlhs_a, "g1s": g1s_a, "g2s": g2s_a, "gfs": gfs_a,
            "wq": wq_a, "wk": wk_a, "wv": wv_a, "wo": wo_a,
            "w1t": w1_a, "w3t": w3_a, "w2t": w2_a, "wlm": wlm_a,
        })
    return scalars, in_maps


def kernel(**inputs) -> np.ndarray:
    from concourse.bass_utils import run_bass_kernel_spmd

    scalars, in_maps = _prep(inputs)
    key = tuple(tuple(s) for s in scalars)
    if key not in _cache:
        _cache[key] = _build(scalars)
    nc = _cache[key]

    import os
    trace = bool(int(os.environ.get("KERNEL_TRACE", "0")))
    res = run_bass_kernel_spmd(nc, in_maps, core_ids=list(range(8)), trace=trace)
    kernel.last_result = res

    logits = np.empty((B, T, V), np.float32)
    for c in range(8):
        b, j = c // 4, c % 4
        logits[b, j * TC:(j + 1) * TC, :] = res.results[c]["logitsT"].T
    return logits



# revision 20
# speedup vs baseline: 1.4662x; 1.4662x over previous
"""BitAstroGPT forward pass on 8 TRN2 NeuronCores.

Sharding: sequence-parallel (4 chunks of 512 tokens per sequence) x
batch-parallel (2 sequences). Core c owns batch c//4, token chunk c%4.
Per layer, two AllGathers (groups [[0..3],[4..7]]) share K (feature-major)
and V (token-major, scaled 1/16); attention runs full-context with
host-provided multiplicative causal masks so the SPMD program is identical
on all cores.

BitNet ternary weights are exact in fp8e4m3; all projections run as
DoubleRow fp8 matmuls with an fp8-residual 2-pass decomposition of the
activations (h ~= fp8(h) + fp8(h - fp8(h))), which keeps bf16-level
accuracy at half the PE cost. The lm_head (not ternary) uses a 3-pass
decomposition with weight+activation residual planes. Per-matrix gamma
scales fold into scalar immediates (exp scale, output scales).
"""
import os
import numpy as np
import ml_dtypes

BF = ml_dtypes.bfloat16
E4 = ml_dtypes.float8_e4m3
E3 = ml_dtypes.float8_e3m4
V, B, T, D, L, H = 32000, 2, 2048, 1024, 4, 16
HD = 64
HID = 2730
HPAD = 2816           # 22 * 128
NMH = HPAD // 128     # 22
TC = 512              # tokens per core
NET = D // 128        # 8 feature tiles
NKT = T // 128        # 16 key tiles
NVT = V // 128        # 250 vocab tiles
EPS = 1e-6
VSCALE = 16.0
GROUPS = [[0, 1, 2, 3], [4, 5, 6, 7]]

# payload dtype for the Q AllGather ("e3" or "bf16")
QDT = os.environ.get("KERNEL_QDT", "e3")
YR = 16 * 65  # y-partial rows (16 heads x (64 dims + denom))

_cache = {}


def _quant(w):
    gamma = max(np.float32(np.mean(np.abs(w), dtype=np.float32)), np.float32(1e-5))
    tern = np.clip(np.round(np.float32(w) / gamma), -1.0, 1.0).astype(np.float32)
    return tern, float(gamma)


def _rope_tables():
    inv_freq = 1.0 / (10000.0 ** (np.arange(0, HD, 2, dtype=np.float32) / HD))
    t = np.arange(T, dtype=np.float32)
    freqs = np.einsum("i,j->ij", t, inv_freq)
    emb = np.concatenate([freqs, freqs], axis=-1)  # [T, 64]
    return np.cos(emb).astype(np.float32), np.sin(emb).astype(np.float32)


def _rot_lhs():
    # rot(q) = M @ q per 64-block; lhsT[e_in, e_out] = M[e_out, e_in]
    M = np.zeros((128, 128), np.float32)
    for blk in range(2):
        o = blk * 64
        for j in range(32):
            M[o + j, o + j + 32] = -1.0
            M[o + j + 32, o + j] = 1.0
    return np.ascontiguousarray(M.T).astype(BF)


def _build(scalars):
    G = int(os.environ.get("KERNEL_G", "4"))
    ACCB = int(os.environ.get("KERNEL_ACCB", "4"))
    SB_ = int(os.environ.get("KERNEL_SB", "3"))
    YB = int(os.environ.get("KERNEL_YB", "1"))
    P8POOL = bool(int(os.environ.get("KERNEL_P8POOL", "1")))
    import concourse.bacc as bacc
    import concourse.mybir as mybir
    import concourse.tile as tile

    F32 = mybir.dt.float32
    BF16 = mybir.dt.bfloat16
    FP8 = mybir.dt.float8e4
    QD = mybir.dt.float8e3 if QDT == "e3" else BF16
    AF = mybir.ActivationFunctionType
    OP = mybir.AluOpType
    DR = mybir.MatmulPerfMode.DoubleRow
    es_l, vo_l, sil_l, m23_l = scalars

    nc = bacc.Bacc("TRN2", target_bir_lowering=False, debug=False, num_devices=8)

    xT0 = nc.dram_tensor("xT0", [D, TC], F32, kind="ExternalInput")
    cosf = nc.dram_tensor("cosf", [128, TC], BF16, kind="ExternalInput")
    sinf = nc.dram_tensor("sinf", [128, TC], BF16, kind="ExternalInput")
    maskT = nc.dram_tensor("maskT", [TC, T], BF16, kind="ExternalInput")
    rlhs = nc.dram_tensor("rlhs", [128, 128], BF16, kind="ExternalInput")
    g1s = nc.dram_tensor("g1s", [128, L * NET], F32, kind="ExternalInput")
    g2s = nc.dram_tensor("g2s", [128, L * NET], F32, kind="ExternalInput")
    gfs = nc.dram_tensor("gfs", [128, NET], F32, kind="ExternalInput")
    wq = nc.dram_tensor("wq", [L, D, D], FP8, kind="ExternalInput")
    wk = nc.dram_tensor("wk", [L, D, D], FP8, kind="ExternalInput")
    wv = nc.dram_tensor("wv", [L, 128, NET, D], FP8, kind="ExternalInput")
    wo = nc.dram_tensor("wo", [L, D, D], FP8, kind="ExternalInput")
    w1t = nc.dram_tensor("w1t", [L, D, HPAD], FP8, kind="ExternalInput")
    w3t = nc.dram_tensor("w3t", [L, D, HPAD], FP8, kind="ExternalInput")
    w2t = nc.dram_tensor("w2t", [L, HPAD, D], FP8, kind="ExternalInput")
    wlm = nc.dram_tensor("wlm", [D, V], FP8, kind="ExternalInput")
    wlmr = nc.dram_tensor("wlmr", [D, V], FP8, kind="ExternalInput")
    logitsT = nc.dram_tensor("logitsT", [V, TC], F32, kind="ExternalOutput")

    with tile.TileContext(nc) as tc:
        with (
            tc.tile_pool(name="sb", bufs=3) as sb,
            tc.tile_pool(name="ps", bufs=ACCB, space="PSUM") as ps,
            tc.tile_pool(name="dram", bufs=1, space="DRAM") as dram,
        ):
            # ---- persistent constants ----
            ones_bf = sb.tile([128, 128], BF16, tag="ones", name="ones_bf", bufs=1)
            nc.vector.memset(ones_bf[:], 1.0)
            ones32 = sb.tile([128, 128], F32, tag="ones32", name="ones32", bufs=1)
            nc.vector.memset(ones32[:], 1.0)
            rlhs_sb = sb.tile([128, 128], BF16, tag="rlhs", name="rlhs_sb", bufs=1)
            nc.sync.dma_start(rlhs_sb[:], rlhs[:])
            cos_sb = sb.tile([128, TC], BF16, tag="cos", name="cos_sb", bufs=1)
            nc.sync.dma_start(cos_sb[:], cosf[:])
            sin_sb = sb.tile([128, TC], BF16, tag="sin", name="sin_sb", bufs=1)
            nc.sync.dma_start(sin_sb[:], sinf[:])
            mask_sb = sb.tile([128, 4, T], BF16, tag="mask", name="mask_sb",
                              bufs=1)
            nc.sync.dma_start(
                mask_sb[:], maskT[:].rearrange("(kt p) t -> p kt t", p=128))
            g1_sb = sb.tile([128, L * NET], F32, tag="g1", name="g1_sb", bufs=1)
            nc.sync.dma_start(g1_sb[:], g1s[:])
            g2_sb = sb.tile([128, L * NET], F32, tag="g2", name="g2_sb", bufs=1)
            nc.sync.dma_start(g2_sb[:], g2s[:])
            gf_sb = sb.tile([128, NET], F32, tag="gf", name="gf_sb", bufs=1)
            nc.sync.dma_start(gf_sb[:], gfs[:])

            eps_sb = sb.tile([1, 1], F32, tag="eps", name="eps_sb", bufs=1)
            nc.vector.memset(eps_sb[:], EPS)

            x_big = sb.tile([128, NET, TC], F32, tag="x", name="x_big", bufs=1)
            for i in range(NET):
                nc.sync.dma_start(x_big[:, i, :], xT0[i * 128:(i + 1) * 128, :])

            k_own = sb.tile([128, NET, TC], BF16, tag="kown", name="k_own",
                            bufs=1)
            q_all = sb.tile([128, 4, NET, TC], QD, tag="qall", name="q_all",
                            bufs=1)
            v_own = sb.tile([128, 4, 16 * 65], BF16, tag="vown", name="v_own",
                            bufs=1)
            ones_view = v_own[:].rearrange("p kt (h c) -> p kt h c", c=65)[:, :, :, 64:65]
            nc.vector.memset(ones_view, 1.0)
            y_fin = sb.tile([65, 16, TC], BF16, tag="yfin", name="y_fin",
                            bufs=1)

            # fp8 activation pair buffers
            h8 = sb.tile([128, NET, TC], FP8, tag="h8", name="h8", bufs=1)
            hr8 = sb.tile([128, NET, TC], FP8, tag="hr8", name="hr8", bufs=1)
            y8 = sb.tile([128, NET, TC], FP8, tag="y8", name="y8", bufs=1)
            yr8 = sb.tile([128, NET, TC], FP8, tag="yr8", name="yr8", bufs=1)
            m8 = sb.tile([128, NMH, TC], FP8, tag="m8", name="m8", bufs=1)
            mr8 = sb.tile([128, NMH, TC], FP8, tag="mr8", name="mr8", bufs=1)

            # ---- helpers ----
            def to_pair8(src_bf, a8_ap, r8_ap):
                """fp8 + residual decomposition of a bf16 tile."""
                if P8POOL:
                    nc.gpsimd.tensor_copy(a8_ap, src_bf)
                else:
                    nc.vector.tensor_copy(a8_ap, src_bf)
                nc.vector.tensor_sub(r8_ap, src_bf, a8_ap)

            def projDR(wslice, pair, n_m, epi, nk=None):
                """out[m,:] = sum_kp sum_pass w(kp)[:, m].T @ pair[pass][kp]

                Weight DMA covers a G-wide m-block; PSUM accs run in
                sub-groups of 2 so two sub-groups pipeline in the acc ring.
                """
                if nk is None:
                    nk = NET
                npair = nk // 2
                npass = len(pair)
                KC = 8  # k-tiles per weight DMA

                def wslice2(k0, kn, g0, gm):
                    return wslice(k0, kn, g0, gm).rearrange(
                        "(k p) m -> p k m", p=128)
                for g0 in range(0, n_m, G):
                    gm = min(G, n_m - g0)
                    w_sbs = []
                    for k0 in range(0, nk, KC):
                        kn = min(KC, nk - k0)
                        w_sb = sb.tile([128, kn, gm * 128], FP8, tag="w",
                                       name="w_sb", bufs=3)
                        nc.sync.dma_start(w_sb[:], wslice2(k0, kn, g0, gm))
                        w_sbs.append((k0, kn, w_sb))
                    for s0 in range(0, gm, 2):
                        sm = min(2, gm - s0)
                        accs = [ps.tile([128, TC], F32, tag="acc",
                                        name=f"acc{mi}") for mi in range(sm)]
                        for (k0, kn, w_sb) in w_sbs:
                            for kp2 in range(kn // 2):
                                kp = k0 // 2 + kp2
                                for mi in range(sm):
                                    lt = w_sb[:, 2 * kp2:2 * kp2 + 2,
                                              (s0 + mi) * 128:
                                              (s0 + mi + 1) * 128]
                                    for pi in range(npass):
                                        nc.tensor.matmul(
                                            accs[mi][:], lt,
                                            pair[pi][:, 2 * kp:2 * kp + 2, :],
                                            start=(kp == 0 and pi == 0),
                                            stop=(kp == npair - 1
                                                  and pi == npass - 1),
                                            perf_mode=DR)
                        for mi in range(sm):
                            epi(g0 + s0 + mi, accs[mi])

            def rmsnorm8(g_base, g_off, a8, r8):
                ssum = ps.tile([1, TC], F32, tag="acc", name="ssum")
                for i in range(NET):
                    x2 = sb.tile([128, TC], BF16, tag="x2", name="x2", bufs=2)
                    nc.vector.tensor_mul(x2[:], x_big[:, i, :], x_big[:, i, :])
                    nc.tensor.matmul(ssum[:], ones_bf[:, 0:1], x2[:],
                                     start=(i == 0), stop=(i == NET - 1))
                sq = sb.tile([1, TC], F32, tag="nrm", name="sq", bufs=2)
                nc.scalar.activation(sq[:], ssum[:], AF.Sqrt, bias=eps_sb[0:1, 0:1],
                                     scale=1.0 / D)
                inv = sb.tile([1, TC], F32, tag="nrm", name="inv", bufs=2)
                nc.vector.reciprocal(inv[:], sq[:])
                rsig = ps.tile([128, TC], F32, tag="acc", name="rsig")
                nc.tensor.matmul(rsig[:], ones32[0:1, :], inv[:], start=True,
                                 stop=True)
                for i in range(NET):
                    o = sb.tile([128, TC], BF16, tag="hn", name="hn", bufs=4)
                    nc.vector.scalar_tensor_tensor(
                        o[:], x_big[:, i, :], g_base[:, g_off + i:g_off + i + 1],
                        rsig[:], OP.mult, OP.mult)
                    to_pair8(o[:], a8[:, i, :], r8[:, i, :])

            def rope_tile(src, sink):
                rp = ps.tile([128, TC], F32, tag="acc", name="rotp")
                nc.tensor.matmul(rp[:], rlhs_sb[:], src[:], start=True, stop=True)
                t1 = sb.tile([128, TC], BF16, tag="rt", name="rt1", bufs=2)
                nc.vector.tensor_mul(t1[:], src[:], cos_sb[:])
                t2 = sb.tile([128, TC], BF16, tag="rt", name="rt2", bufs=2)
                nc.vector.tensor_mul(t2[:], rp[:], sin_sb[:])
                return sink(t1, t2)

            # ---- layers ----
            for l in range(L):
                rmsnorm8(g1_sb, l * NET, h8, hr8)

                ag_inq = dram.tile([D, TC], QD, tag="agiq", name="ag_inq")
                agq_outs = [dram.tile([4 * 512, TC], QD, tag=f"agqo{i}",
                                      name=f"agq_out{i}") for i in range(2)]
                ag_inys = [dram.tile([4 * 520, TC], BF16, tag=f"agiy{i}",
                                     name=f"ag_iny{i}") for i in range(2)]
                rs_outs = [dram.tile([520, TC], BF16, tag=f"rso{i}",
                                     name=f"rs_out{i}") for i in range(2)]

                # q projection with fused rope (q/16 -> fp8e3 payload),
                # in head-halves; the AllGather for each half fires as soon
                # as that half is projected so gather overlaps k/v work
                def q_epi(m, acc):
                    t = sb.tile([128, TC], BF16, tag="qsb", name="qsb", bufs=3)
                    nc.any.tensor_copy(t[:], acc[:])
                    def sink(t1, t2, _m=m):
                        qr = sb.tile([128, TC], QD, tag="qr", name="qr", bufs=3)
                        nc.vector.tensor_add(qr[:], t1[:], t2[:])
                        nc.sync.dma_start(ag_inq[_m * 128:(_m + 1) * 128, :],
                                          qr[:])
                    rope_tile(t, sink)
                for qh in range(2):
                    projDR(lambda k0, kn, g0, gm, _l=l, _qh=qh:
                           wq[_l, k0 * 128:(k0 + kn) * 128,
                              (_qh * 4 + g0) * 128:(_qh * 4 + g0 + gm) * 128],
                           (h8, hr8), 4,
                           lambda m, acc, _qh=qh: q_epi(_qh * 4 + m, acc))
                    nc.gpsimd.collective_compute(
                        "AllGather", mybir.AluOpType.bypass,
                        replica_groups=GROUPS,
                        ins=[ag_inq[qh * 512:(qh + 1) * 512, :]],
                        outs=[agq_outs[qh][:]])
                    for r in range(4):
                        nc.sync.dma_start(
                            q_all[:, r, qh * 4:(qh + 1) * 4, :],
                            agq_outs[qh][r * 512:(r + 1) * 512, :]
                            .rearrange("(e p) t -> p e t", p=128))

                # k projection (own chunk, bf16, fused rope) -> k_own
                def k_epi(m, acc):
                    t = sb.tile([128, TC], BF16, tag="ksb", name="ksb", bufs=3)
                    nc.any.tensor_copy(t[:], acc[:])
                    def sink(t1, t2, _m=m):
                        nc.vector.tensor_add(k_own[:, _m, :], t1[:], t2[:])
                    rope_tile(t, sink)
                projDR(lambda k0, kn, g0, gm, _l=l: wk[_l, k0 * 128:(k0 + kn) * 128,
                                                      g0 * 128:(g0 + gm) * 128],
                       (h8, hr8), NET, k_epi)

                # v projection, token-major, own chunk -> v_own (bf16)
                for half in range(2):
                    wv_sb = sb.tile([128, NET, TC], FP8, tag="w",
                                    name="wv_sb", bufs=3)
                    nc.sync.dma_start(
                        wv_sb[:], wv[l, :, :, half * 512:(half + 1) * 512])
                    for t0 in range(0, 4, 2):
                        vaccs = [ps.tile([128, TC], F32, tag="acc",
                                         name=f"vacc{tt}") for tt in range(2)]
                        for kp in range(NET // 2):
                            for tt in range(2):
                                for pi, buf in enumerate((h8, hr8)):
                                    nc.tensor.matmul(
                                        vaccs[tt][:],
                                        buf[:, 2 * kp:2 * kp + 2,
                                            (t0 + tt) * 128:
                                            (t0 + tt + 1) * 128],
                                        wv_sb[:, 2 * kp:2 * kp + 2, :],
                                        start=(kp == 0 and pi == 0),
                                        stop=(kp == NET // 2 - 1 and pi == 1),
                                        perf_mode=DR)
                        for tt in range(2):
                            # strided copy into the 65-wide per-head v slots
                            dst = v_own[:, t0 + tt, :].rearrange(
                                "p (h c) -> p h c", c=65)[:, half * 8:
                                                          half * 8 + 8, 0:64]
                            nc.any.tensor_copy(dst, vaccs[tt][:].rearrange(
                                "p (h c) -> p h c", c=64))

                # partial attention: own 512 keys x all 2048 queries
                for h in range(16):
                    et, base = h // 2, (h % 2) * 64
                    for qb in range(4):
                        y_aug = ps.tile([65, TC], F32, tag="y", name="y_aug",
                                        bufs=YB)
                        for k2 in range(2):
                            s2 = ps.tile([128, 2, TC], F32, tag="s", name="s2",
                                         bufs=SB_)
                            for ki in range(2):
                                kt = 2 * k2 + ki
                                nc.tensor.matmul(
                                    s2[:, ki, :],
                                    k_own[base:base + 64, et,
                                          kt * 128:(kt + 1) * 128],
                                    q_all[base:base + 64, qb, et, :],
                                    start=True, stop=True)
                            p_sb = sb.tile([128, 2, TC], BF16, tag="p",
                                           name="p_sb", bufs=5)
                            nc.scalar.activation(p_sb[:], s2[:], AF.Exp,
                                                 scale=es_l[l])
                            nc.vector.tensor_mul(
                                p_sb[:], p_sb[:],
                                mask_sb[:, 2 * k2:2 * k2 + 2,
                                        qb * TC:(qb + 1) * TC])
                            for ki in range(2):
                                kt = 2 * k2 + ki
                                nc.tensor.matmul(
                                    y_aug[:],
                                    v_own[:, kt, h * 65:(h + 1) * 65],
                                    p_sb[:, ki, :],
                                    start=(kt == 0), stop=(kt == 3))
                        yst = sb.tile([65, TC], BF16, tag="yst", name="yst",
                                      bufs=3)
                        nc.any.tensor_copy(yst[:], y_aug[:])
                        nc.sync.dma_start(
                            ag_inys[h // 8][qb * 520 + (h % 8) * 65:
                                            qb * 520 + (h % 8 + 1) * 65, :],
                            yst[:])

                # reduce-scatter the y partials per head-half; block qb
                # goes to rank qb (rows within a block: h-major, p-minor)
                for hh in range(2):
                    nc.gpsimd.collective_compute(
                        "ReduceScatter", mybir.AluOpType.add,
                        replica_groups=GROUPS,
                        ins=[ag_inys[hh][:]], outs=[rs_outs[hh][:]])
                    nc.sync.dma_start(
                        y_fin[:, hh * 8:(hh + 1) * 8, :],
                        rs_outs[hh][:]
                        .rearrange("(h p) t -> p h t", p=65))

                # normalize y = y_aug / denom per head, then fp8 pair per et
                for et in range(NET):
                    yt = sb.tile([128, TC], BF16, tag="yt", name="yt", bufs=3)
                    for half in range(2):
                        h = 2 * et + half
                        rec = sb.tile([1, TC], F32, tag="rec", name="rec",
                                      bufs=2)
                        nc.vector.reciprocal(rec[0:1, :], y_fin[64:65, h, :])
                        rh_sb = sb.tile([64, TC], F32, tag="rh", name="rh_sb",
                                        bufs=2)
                        nc.gpsimd.partition_broadcast(rh_sb[:], rec[0:1, :])
                        nc.vector.tensor_mul(
                            yt[half * 64:(half + 1) * 64, :],
                            y_fin[0:64, h, :], rh_sb[:])
                    to_pair8(yt[:], y8[:, et, :], yr8[:, et, :])

                def o_epi(m, acc, _l=l):
                    nc.vector.scalar_tensor_tensor(
                        x_big[:, m, :], acc[:], vo_l[_l], x_big[:, m, :],
                        OP.mult, OP.add)
                projDR(lambda k0, kn, g0, gm, _l=l: wo[_l, k0 * 128:(k0 + kn) * 128,
                                                      g0 * 128:(g0 + gm) * 128],
                       (y8, yr8), NET, o_epi)

                # ---- MLP ----
                rmsnorm8(g2_sb, l * NET, h8, hr8)
                for g0 in range(0, NMH, G):
                    gm = min(G, NMH - g0)
                    s_tiles, b_tiles = [], []
                    def s_epi(m, acc, _l=l):
                        t = sb.tile([128, TC], BF16, tag="asb", name="asb",
                                    bufs=G + 1)
                        nc.scalar.activation(t[:], acc[:], AF.Silu,
                                             scale=sil_l[_l])
                        s_tiles.append(t)
                    def b_epi(m, acc):
                        t = sb.tile([128, TC], BF16, tag="bsb", name="bsb",
                                    bufs=G + 1)
                        nc.any.tensor_copy(t[:], acc[:])
                        b_tiles.append(t)
                    projDR(lambda k0, kn, gg0, gm_, _l=l, _g0=g0:
                           w1t[_l, k0 * 128:(k0 + kn) * 128,
                               (_g0 + gg0) * 128:(_g0 + gg0 + gm_) * 128],
                           (h8, hr8), gm, s_epi)
                    projDR(lambda k0, kn, gg0, gm_, _l=l, _g0=g0:
                           w3t[_l, k0 * 128:(k0 + kn) * 128,
                               (_g0 + gg0) * 128:(_g0 + gg0 + gm_) * 128],
                           (h8, hr8), gm, b_epi)
                    for mi in range(gm):
                        pr = sb.tile([128, TC], BF16, tag="prod", name="prod",
                                     bufs=4)
                        nc.vector.tensor_mul(pr[:], s_tiles[mi][:], b_tiles[mi][:])
                        to_pair8(pr[:], m8[:, g0 + mi, :], mr8[:, g0 + mi, :])

                def w2_epi(m, acc, _l=l):
                    nc.vector.scalar_tensor_tensor(
                        x_big[:, m, :], acc[:], m23_l[_l], x_big[:, m, :],
                        OP.mult, OP.add)
                projDR(lambda k0, kn, g0, gm, _l=l: w2t[_l, k0 * 128:(k0 + kn) * 128,
                                                       g0 * 128:(g0 + gm) * 128],
                       (m8, mr8), NET, w2_epi, nk=NMH)

            # ---- final norm + lm head (3-pass fp8: w8.h8 + w8.r8 + wr8.h8) --
            rmsnorm8(gf_sb, 0, h8, hr8)

            for g0 in range(0, NVT, 4):
                gm = min(4, NVT - g0)
                w_sb = sb.tile([128, NET, gm * 128], FP8, tag="w",
                               name="lw_sb", bufs=3)
                nc.sync.dma_start(
                    w_sb[:], wlm[:, g0 * 128:(g0 + gm) * 128]
                    .rearrange("(k p) m -> p k m", p=128))
                wr_sb = sb.tile([128, NET, gm * 128], FP8, tag="w",
                                name="lwr_sb", bufs=3)
                nc.sync.dma_start(
                    wr_sb[:], wlmr[:, g0 * 128:(g0 + gm) * 128]
                    .rearrange("(k p) m -> p k m", p=128))
                lg4 = sb.tile([128, gm, TC], F32, tag="lg", name="lg4", bufs=2)
                for s0 in range(0, gm, 2):
                    sm = min(2, gm - s0)
                    accs = [ps.tile([128, TC], F32, tag="acc",
                                    name=f"lacc{mi}") for mi in range(sm)]
                    for kp in range(NET // 2):
                        for mi in range(sm):
                            mj = (s0 + mi) * 128
                            lt = w_sb[:, 2 * kp:2 * kp + 2, mj:mj + 128]
                            ltr = wr_sb[:, 2 * kp:2 * kp + 2, mj:mj + 128]
                            hs = h8[:, 2 * kp:2 * kp + 2, :]
                            rs = hr8[:, 2 * kp:2 * kp + 2, :]
                            nc.tensor.matmul(accs[mi][:], lt, hs,
                                             start=(kp == 0), stop=False,
                                             perf_mode=DR)
                            nc.tensor.matmul(accs[mi][:], lt, rs,
                                             start=False, stop=False,
                                             perf_mode=DR)
                            nc.tensor.matmul(accs[mi][:], ltr, hs,
                                             start=False,
                                             stop=(kp == NET // 2 - 1),
                                             perf_mode=DR)
                    for mi in range(sm):
                        nc.scalar.activation(lg4[:, s0 + mi, :], accs[mi][:],
                                             AF.Copy, scale=1.0 / VSCALE)
                nc.sync.dma_start(
                    logitsT[g0 * 128:(g0 + gm) * 128, :]
                    .rearrange("(m p) t -> p m t", p=128), lg4[:])

    nc.compile()
    return nc


def _prep(inputs):
    """Host-side prep: quantization, layouts, per-core in_maps."""
    idx = np.asarray(inputs["idx"])
    emb = np.asarray(inputs["emb"], np.float32)

    qw = {}
    gam = {}
    for name in ["Wq", "Wk", "Wv", "Wo", "W1", "W3", "W2"]:
        W = np.asarray(inputs[name], np.float32)
        qw[name] = []
        gam[name] = []
        for l in range(L):
            t, g = _quant(W[l])
            qw[name].append(t)
            gam[name].append(g)

    qsc = VSCALE if QDT == "e3" else 1.0
    es_l = tuple(gam["Wq"][l] * gam["Wk"][l] * qsc / np.sqrt(HD)
                 for l in range(L))
    vo_l = tuple(gam["Wv"][l] * gam["Wo"][l] for l in range(L))
    sil_l = tuple(gam["W1"][l] for l in range(L))
    m23_l = tuple(gam["W2"][l] * gam["W3"][l] for l in range(L))
    scalars = (es_l, vo_l, sil_l, m23_l)

    # weight arrays, fp8e4 ternary, lhsT layout [K=in_feat, M=out_feat]
    wq_a = np.stack([(qw["Wq"][l].T / qsc) for l in range(L)]).astype(E4)
    wk_a = np.stack([qw["Wk"][l].T for l in range(L)]).astype(E4)
    wo_a = np.stack([qw["Wo"][l].T for l in range(L)]).astype(E4)
    # V weights: rhs layout [128, NET, D]
    wv_a = np.stack([
        qw["Wv"][l].T.reshape(NET, 128, D).transpose(1, 0, 2)
        for l in range(L)]).astype(E4)
    w1_a = np.zeros((L, D, HPAD), E4)
    w3_a = np.zeros((L, D, HPAD), E4)
    w2_a = np.zeros((L, HPAD, D), E4)
    for l in range(L):
        w1_a[l, :, :HID] = qw["W1"][l].T.astype(E4)
        w3_a[l, :, :HID] = qw["W3"][l].T.astype(E4)
        w2_a[l, :HID, :] = qw["W2"][l].T.astype(E4)
    wlm_f = np.ascontiguousarray(np.asarray(inputs["Wlm"], np.float32).T) * VSCALE
    wlm_a = wlm_f.astype(E4)
    wlmr_a = (wlm_f - wlm_a.astype(np.float32)).astype(E4)

    def gcol(g):  # [L, D] -> [128, L*8]
        return np.ascontiguousarray(
            np.asarray(g, np.float32).reshape(-1, NET, 128).transpose(2, 0, 1)
            .reshape(128, -1))
    g1s_a = gcol(inputs["g1"])
    g2s_a = gcol(inputs["g2"])
    gfs_a = gcol(np.asarray(inputs["gf"], np.float32)[None])
    rlhs_a = _rot_lhs()

    cos, sin = _rope_tables()
    row = np.tile(np.arange(HD), 2)

    in_maps = []
    for c in range(8):
        b, j = c // 4, c % 4
        tsl = slice(j * TC, (j + 1) * TC)
        toks = idx[b, tsl]
        x0 = np.ascontiguousarray(emb[toks].T)  # [D, TC] f32
        cos_fm = np.ascontiguousarray(cos[tsl][:, row].T).astype(BF)
        sin_fm = np.ascontiguousarray(sin[tsl][:, row].T).astype(BF)
        # [own keys, all queries]: visible iff k_abs <= q_abs
        tk = np.arange(j * TC, (j + 1) * TC)[:, None]
        tq = np.arange(T)[None, :]
        mask = (tk <= tq).astype(np.float32).astype(BF)
        in_maps.append({
            "xT0": x0, "cosf": cos_fm, "sinf": sin_fm, "maskT": mask,
            "rlhs": rlhs_a, "g1s": g1s_a, "g2s": g2s_a, "gfs": gfs_a,
            "wq": wq_a, "wk": wk_a, "wv": wv_a, "wo": wo_a,
            "w1t": w1_a, "w3t": w3_a, "w2t": w2_a, "wlm": wlm_a,
            "wlmr": wlmr_a,
        })
    return scalars, in_maps


def kernel(**inputs) -> np.ndarray:
    from concourse.bass_utils import run_bass_kernel_spmd

    scalars, in_maps = _prep(inputs)
    key = tuple(tuple(s) for s in scalars)
    if key not in _cache:
        _cache[key] = _build(scalars)
    nc = _cache[key]

    trace = bool(int(os.environ.get("KERNEL_TRACE", "0")))
    res = run_bass_kernel_spmd(nc, in_maps, core_ids=list(range(8)), trace=trace)
    kernel.last_result = res

    logits = np.empty((B, T, V), np.float32)
    for c in range(8):
        b, j = c // 4, c % 4
        logits[b, j * TC:(j + 1) * TC, :] = res.results[c]["logitsT"].T
    return logits
